# revision 14
# baseline (speedup 1.0000x reference)
"""CascadedGroupAttention Trainium2 kernel.

Data-parallel over batch: B=512 split as 64 samples x 8 cores. Inside each
core a fully fused per-head cascade runs phase-major over sample blocks.

Key restructurings vs the reference:
  - qkv BN affine folded into matmul weights; bias applied via a ones-row
    appended to the feat operand (K=65).
  - softmax 1/sqrt(d) scale folded into the k weights.
  - depthwise 5x5 conv computed on the tensor engine as 25 PSUM-accumulated
    matmuls with [k;q]-interleaved diagonal weight matrices and
    edge-trimmed access patterns (no padding, no im2col).
  - attention computed transposed (P^T = K^T Q + ab^T) so softmax needs no
    transposes: raw exp is safe (logits bounded ~[-9, 10]), the denominator
    comes from a ones-column appended to v^T in the AV matmul, and the
    1/denom broadcast across partitions is done by gpsimd.
  - relative-position bias added by an identity-weight matmul accumulating
    onto the QK PSUM bank (ab is symmetric, so ab^T = ab).
  - relu'd head outputs collected in bf16; the output projection runs in
    bf16 with its BN affine folded into weights/final eviction bias.
"""

import os
import sys

import numpy as np

sys.path.insert(0, "/opt/trn_rl_repo")

import concourse.bass as bass  # noqa: E402
from concourse import bacc  # noqa: E402
import concourse.mybir as mybir  # noqa: E402
from concourse.masks import make_identity  # noqa: E402
from concourse.tile import TileContext  # noqa: E402

F32 = mybir.dt.float32
BF16 = mybir.dt.bfloat16

NHEADS = 4
KD = 16          # key dim
DV = 64          # per-head value dim
CH = 64          # per-head input channels (dim // heads)
RES = 14
N = RES * RES    # 196 tokens
DIM = 256
BATCH = 512
NCORES = 8
SPC = BATCH // NCORES   # samples per core = 64
BLK = 16                # samples per pipeline block
SCALE = KD ** -0.5

# tap order: center first so the first conv matmul covers the full output
# region (start=True then has full has_written coverage for accumulation)
TAPS = [(0, 0)] + [
    (dr, dc) for dr in range(-2, 3) for dc in range(-2, 3) if (dr, dc) != (0, 0)
]


def _prep_host(inp):
    """Fold BN affines into weights and build hardware-layout arrays."""
    import ml_dtypes

    bf16 = ml_dtypes.bfloat16
    qkv_w = np.asarray(inp["qkv_w"], np.float32)
    qkv_scale = np.asarray(inp["qkv_scale"], np.float32)
    qkv_bias = np.asarray(inp["qkv_bias"], np.float32)
    dw_w = np.asarray(inp["dw_w"], np.float32)
    dw_scale = np.asarray(inp["dw_scale"], np.float32)
    dw_bias = np.asarray(inp["dw_bias"], np.float32)
    proj_w = np.asarray(inp["proj_w"], np.float32)
    proj_scale = np.asarray(inp["proj_scale"], np.float32)
    proj_bias = np.asarray(inp["proj_bias"], np.float32)
    ab_full = np.asarray(inp["attention_biases"], np.float32)[
        :, np.asarray(inp["bias_idxs"])
    ]  # [4, 196, 196], symmetric in (n, m)

    # fused q+v per-head weight: cols 0:16 q, 16:64 zero, 64:128 v (v at 64
    # so its PE transpose reads a legal 64-aligned base). k separate, M=32
    # (cols 16:32 zero), col-tiled into 32-row slots as in the 4-sample
    # scheme. Row 64 carries the folded BN bias (ones-row trick).
    w_qv = np.zeros((NHEADS, 65, 128), np.float32)
    w_k = np.zeros((NHEADS, 65, 32), np.float32)
    conv_diag = np.zeros((NHEADS, 25, 128, 128), np.float32)
    dwb_pat = np.zeros((NHEADS, 128, 1), np.float32)
    for i in range(NHEADS):
        for j in range(KD):
            w_qv[i, :CH, j] = qkv_w[i, j] * qkv_scale[i, j]
            w_qv[i, CH, j] = qkv_bias[i, j]
            w_k[i, :CH, j] = qkv_w[i, KD + j] * qkv_scale[i, KD + j] * SCALE
            w_k[i, CH, j] = qkv_bias[i, KD + j] * SCALE
        for d in range(DV):
            w_qv[i, :CH, 64 + d] = qkv_w[i, 2 * KD + d] * qkv_scale[i, 2 * KD + d]
            w_qv[i, CH, 64 + d] = qkv_bias[i, 2 * KD + d]
        for t, (dr, dc) in enumerate(TAPS):
            for p in range(128):
                c = p % 32
                if c < 16:  # q channels sit in rows 32j..32j+15
                    conv_diag[i, t, p, p] = dw_w[i, c, dr + 2, dc + 2] * dw_scale[i, c]
        for p in range(128):
            c = p % 32
            if c < 16:
                dwb_pat[i, p, 0] = dw_bias[i, c]
    # scatter: route q (rows 0:16 of a qv pair tile) to rows 32j..32j+16
    sctr32 = np.zeros((16, 4, 128), np.float32)
    for c in range(16):
        for j in range(4):
            sctr32[c, j, 32 * j + c] = 1.0

    proj_wT = np.ascontiguousarray(
        (proj_w * proj_scale[:, None]).T
    )  # [cat_c, out_o]
    pw_bf = proj_wT.astype(bf16)
    pb = np.ascontiguousarray(proj_bias.reshape(2, 128, 1).astype(np.float32))

    return {
        "w_qv": w_qv.astype(bf16),
        "w_k": w_k.astype(bf16),
        "sctr32": sctr32.astype(bf16),
        "conv_diag": np.ascontiguousarray(conv_diag.astype(bf16)),
        "dwb_pat": dwb_pat,
        "ab": np.ascontiguousarray(ab_full.astype(bf16)),
        "proj_wT": pw_bf,
        "proj_b": pb,
    }


def build_bass(spc=SPC, blk=BLK):
    nc = bacc.Bacc(None, target_bir_lowering=False)

    x_d = nc.declare_dram_parameter("x", [spc, DIM, N], BF16, isOutput=False)
    wqv_d = nc.declare_dram_parameter("w_qv", [NHEADS, 65, 128], BF16, isOutput=False)
    wk_d = nc.declare_dram_parameter("w_k", [NHEADS, 65, 32], BF16, isOutput=False)
    sctr_d = nc.declare_dram_parameter("sctr32", [16, 4, 128], BF16, isOutput=False)
    cdiag_d = nc.declare_dram_parameter(
        "conv_diag", [NHEADS, 25, 128, 128], BF16, isOutput=False
    )
    dwb_d = nc.declare_dram_parameter("dwb_pat", [NHEADS, 128, 1], F32, isOutput=False)
    ab_d = nc.declare_dram_parameter("ab", [NHEADS, N, N], BF16, isOutput=False)
    pw_d = nc.declare_dram_parameter("proj_wT", [DIM, DIM], BF16, isOutput=False)
    pb_d = nc.declare_dram_parameter("proj_b", [2, 128, 1], F32, isOutput=False)
    out_d = nc.declare_dram_parameter("out", [spc, DIM, N], F32, isOutput=True)

    nblk = spc // blk
    npair = blk // 2

    with TileContext(nc) as tc:
        with (
            tc.tile_pool(name="const", bufs=1) as constp,
            tc.tile_pool(name="persist", bufs=1) as persist,
            tc.tile_pool(name="work", bufs=3) as work,
            tc.tile_pool(name="outp", bufs=4) as outp,
        ):
            # ---- constants ----
            ident = constp.tile([128, 128], BF16, name="ident")
            make_identity(nc, ident)
            sctr_sb = constp.tile([16, 4, 128], BF16, name="sctr")
            nc.sync.dma_start(out=sctr_sb, in_=sctr_d[0:16])
            wqv_sb = []
            wk_sb = []
            dwb_sb = []
            ab0_sb = []
            ab1_sb = []
            for i in range(NHEADS):
                t = constp.tile([65, 128], BF16, name=f"wqv{i}", tag=f"wqv{i}")
                nc.sync.dma_start(out=t, in_=wqv_d[i])
                wqv_sb.append(t)
                t = constp.tile([65, 32], BF16, name=f"wk{i}", tag=f"wk{i}")
                nc.sync.dma_start(out=t, in_=wk_d[i])
                wk_sb.append(t)
                t = constp.tile([128, 1], F32, name=f"dwb{i}", tag=f"dwb{i}")
                nc.sync.dma_start(out=t, in_=dwb_d[i])
                dwb_sb.append(t)
                t = constp.tile([128, 2, N], BF16, name=f"ab0_{i}", tag=f"ab0_{i}")
                nc.sync.dma_start(out=t[:, 0, :], in_=ab_d[i, 0:128, :])
                nc.sync.dma_start(out=t[:, 1, :], in_=ab_d[i, 0:128, :])
                ab0_sb.append(t)
                t = constp.tile([68, 2, N], BF16, name=f"ab1_{i}", tag=f"ab1_{i}")
                nc.sync.dma_start(out=t[:, 0, :], in_=ab_d[i, 128:196, :])
                nc.sync.dma_start(out=t[:, 1, :], in_=ab_d[i, 128:196, :])
                ab1_sb.append(t)
            cdg_sb = []
            for i in range(NHEADS):
                t = constp.tile([128, 25, 128], BF16, name=f"cdg{i}", tag=f"cdg{i}")
                nc.sync.dma_start(out=t, in_=cdiag_d[i].rearrange("t p c -> p t c"))
                cdg_sb.append(t)
            pw0 = constp.tile([128, DIM], BF16, name="pw0")
            nc.sync.dma_start(out=pw0, in_=pw_d[0:128, :])
            pw1 = constp.tile([128, DIM], BF16, name="pw1")
            nc.sync.dma_start(out=pw1, in_=pw_d[128:256, :])
            pb0 = constp.tile([128, 1], F32, name="pb0")
            nc.sync.dma_start(out=pb0, in_=pb_d[0])
            pb1 = constp.tile([128, 1], F32, name="pb1")
            nc.sync.dma_start(out=pb1, in_=pb_d[1])

            for b in range(min(nblk, int(os.environ.get('MAXBLK', '99')))):
                s0 = b * blk
                # ---- per-block persistent tiles (sample pairs) ----
                featp = []
                for p in range(npair):
                    t = persist.tile(
                        [65, 392], BF16, name=f"feat{b}_{p}", tag=f"feat{p}", bufs=2
                    )
                    nc.sync.dma_start(
                        out=t[0:CH, :].rearrange("c (s n) -> c s n", s=2),
                        in_=x_d[s0 + 2 * p : s0 + 2 * p + 2, 0:CH, :].rearrange(
                            "s c n -> c s n"
                        ),
                    )
                    nc.gpsimd.memset(t[CH : CH + 1, :], 1.0)
                    featp.append(t)
                rcat01 = []
                rcat23 = []
                for p in range(npair):
                    rcat01.append(
                        persist.tile(
                            [128, 392], BF16, name=f"rA{b}_{p}", tag=f"rA{p}", bufs=2
                        )
                    )
                    rcat23.append(
                        persist.tile(
                            [128, 392], BF16, name=f"rB{b}_{p}", tag=f"rB{p}", bufs=2
                        )
                    )

                for i in range(NHEADS):
                    # prefetch next head's x chunk (pair layout)
                    x_sb = None
                    if i < NHEADS - 1:
                        x_sb = []
                        for p in range(npair):
                            t = work.tile(
                                [CH, 392],
                                BF16,
                                name=f"x{b}_{i}_{p}",
                                tag=f"x{p}",
                                bufs=2,
                            )
                            nc.sync.dma_start(
                                out=t.rearrange("c (s n) -> c s n", s=2),
                                in_=x_d[
                                    s0 + 2 * p : s0 + 2 * p + 2,
                                    (i + 1) * CH : (i + 2) * CH,
                                    :,
                                ].rearrange("s c n -> c s n"),
                            )
                            x_sb.append(t)

                    # ================= phase A =================
                    # per pair: one fused q+v matmul (q rows 0:16, v rows
                    # 64:128); k col-tiled 4 pairs to a bank at rows 32j.
                    qv = []    # per-pair SBUF [128, 392] bf16
                    kf = []    # per-half-block SBUF [128, 392] bf16 (4 pairs)
                    qf = []    # per-half-block conv'd q, same layout as kf
                    vT = []    # per-sample transposed v [128, 130]
                    with tc.tile_pool(name=f"psA{b}_{i}", bufs=1, space="PSUM") as pA:
                        for p in range(npair):
                            qvp = pA.tile([128, 392], F32, name=f"qv{p}", tag="qv",
                                          bufs=2)
                            nc.tensor.matmul(
                                qvp, wqv_sb[i], featp[p], start=True, stop=True
                            )
                            t = persist.tile(
                                [128, 392], BF16, name=f"qv{b}_{i}_{p}",
                                tag=f"qv{p}", bufs=2,
                            )
                            if p % 2 == 0:
                                nc.scalar.copy(t, qvp)
                            else:
                                nc.vector.tensor_copy(t, qvp)
                            qv.append(t)

                        # k matmuls: bank t holds pairs 4t..4t+3 at rows 32j
                        for th in range(2):
                            kp = pA.tile([128, 392], F32, name=f"kp{th}", tag="kp",
                                         bufs=2)
                            for j in range(4):
                                nc.tensor.matmul(
                                    kp[32 * j : 32 * j + 32, :],
                                    wk_sb[i],
                                    featp[4 * th + j],
                                    start=True,
                                    stop=True,
                                    tile_position=(0, 32 * j),
                                )
                            t = persist.tile(
                                [128, 392], BF16, name=f"kf{b}_{i}_{th}",
                                tag=f"kf{th}", bufs=2,
                            )
                            if th == 0:
                                nc.scalar.copy(t, kp)
                            else:
                                nc.vector.tensor_copy(t, kp)
                            kf.append(t)

                        # v transposes (row-groups 2-3) overlap the q scatter
                        for p in range(npair):
                            for e in range(2):
                                sl = 2 * p + e
                                vTp = pA.tile([128, 128], BF16, name=f"vTp{sl}",
                                              tag="vTp", bufs=1)
                                nc.tensor.transpose(
                                    vTp[0:128, 0:64],
                                    qv[p][64:128, 196 * e : 196 * e + 128],
                                    ident[64:128, 64:128],
                                )
                                nc.tensor.transpose(
                                    vTp[0:68, 64:128],
                                    qv[p][64:128, 196 * e + 128 : 196 * e + 196],
                                    ident[64:128, 64:128],
                                )
                                vT_t = persist.tile(
                                    [128, 130], BF16, name=f"vT{b}_{i}_{sl}",
                                    tag=f"vT{sl}", bufs=2,
                                )
                                nc.vector.tensor_copy(vT_t[:, 0:64], vTp[:, 0:64])
                                nc.vector.tensor_copy(
                                    vT_t[0:68, 65:129], vTp[0:68, 64:128]
                                )
                                nc.gpsimd.memset(vT_t[:, 64:65], 1.0)
                                nc.gpsimd.memset(vT_t[:, 129:130], 1.0)
                                vT.append(vT_t)

                        # scatter q into conv layout (rows 32j, pairs 4t..4t+3)
                        # then 8-sample packed conv per half-block
                        for th in range(2):
                            qs = pA.tile([128, 392], F32, name=f"qs{th}", tag="qs",
                                         bufs=2)
                            for j in range(4):
                                nc.tensor.matmul(
                                    qs,
                                    sctr_sb[:, j, :],
                                    qv[4 * th + j][0:16, :],
                                    start=(j == 0),
                                    stop=(j == 3),
                                )
                            qpad = work.tile([128, 648], BF16, name=f"qpad{th}",
                                             tag=f"qpad{th}")
                            nc.gpsimd.memset(qpad, 0.0)
                            qp4 = qpad.rearrange("p (h r c) -> p h r c", h=2, c=18)
                            nc.scalar.copy(
                                qp4[:, :, 2:16, 2:16],
                                qs.rearrange("p (h r c) -> p h r c", h=2, c=RES),
                            )
                            dqp = pA.tile([128, 392], F32, name=f"dqp{th}",
                                          tag="dqp", bufs=1)
                            for t_, (dr, dc) in enumerate(TAPS):
                                nc.tensor.matmul(
                                    dqp,
                                    cdg_sb[i][:, t_, :],
                                    qp4[:, :, 2 + dr : 16 + dr, 2 + dc : 16 + dc],
                                    start=(t_ == 0),
                                    stop=(t_ == len(TAPS) - 1),
                                )
                            g8 = work.tile([128, 392], BF16, name=f"g8{th}",
                                           tag="g8")
                            nc.scalar.activation(
                                g8, dqp, mybir.ActivationFunctionType.Gelu,
                                bias=dwb_sb[i], scale=1.0,
                            )
                            qf_t = persist.tile([128, 392], BF16,
                                                name=f"qf{b}_{i}_{th}",
                                                tag=f"qf{th}", bufs=2)
                            nc.vector.tensor_add(
                                qf_t.rearrange("p (h r c) -> p h r c", h=2, c=RES),
                                g8.rearrange("p (h r c) -> p h r c", h=2, c=RES),
                                qp4[:, :, 2:16, 2:16],
                            )
                            qf.append(qf_t)

                    # ================= phase B =================
                    with tc.tile_pool(name=f"psB{b}_{i}", bufs=1, space="PSUM") as pB:
                        for q4 in range(blk // 4):  # 4 samples at a time
                            av = None
                            for ph in range(2):  # pairs in this quad
                                pT0 = pB.tile(
                                    [128, 512], F32, name=f"pT0_{ph}", tag="pT0",
                                    bufs=2,
                                )
                                pT1 = pB.tile(
                                    [68, 512], F32, name=f"pT1_{ph}", tag="pT1",
                                    bufs=2,
                                )
                                # start=True clears has_written for the WHOLE
                                # bank, so each sample's QK+ab group must fully
                                # complete before the next sample's QK starts.
                                for j2 in range(2):
                                    sl = 4 * q4 + 2 * ph + j2
                                    p, e = sl // 2, sl % 2
                                    th, j = p // 4, p % 4
                                    kf_s = kf[th][
                                        32 * j : 32 * j + 16,
                                        196 * e : 196 * e + 196,
                                    ]
                                    qf_s = qf[th][
                                        32 * j : 32 * j + 16,
                                        196 * e : 196 * e + 196,
                                    ]
                                    off = 196 * j2
                                    nc.tensor.matmul(
                                        pT0[:, off : off + 196],
                                        kf_s[:, 0:128],
                                        qf_s,
                                        start=True,
                                        stop=False,
                                        tile_position=(32 * j, 0),
                                    )
                                    nc.tensor.matmul(
                                        pT1[:, off : off + 196],
                                        kf_s[:, 128:196],
                                        qf_s,
                                        start=True,
                                        stop=False,
                                        tile_position=(32 * j, 0),
                                    )
                                    nc.tensor.matmul(
                                        pT0[:, off : off + 196],
                                        ident,
                                        ab0_sb[i][:, j2, :],
                                        start=False,
                                        stop=True,
                                    )
                                    nc.tensor.matmul(
                                        pT1[:, off : off + 196],
                                        ident[0:68, 0:68],
                                        ab1_sb[i][:, j2, :],
                                        start=False,
                                        stop=True,
                                    )
                                if av is None:
                                    av = pB.tile(
                                        [65, 2048], F32, name=f"av{q4}", tag="av"
                                    )
                                # exp (2 samples per op, contiguous regions)
                                eP0 = work.tile(
                                    [128, 392], BF16, name=f"eP0_{ph}", tag=f"eP0_{ph}"
                                )
                                nc.scalar.activation(
                                    eP0, pT0[:, 0:392],
                                    mybir.ActivationFunctionType.Exp,
                                )
                                eP1 = work.tile(
                                    [68, 392], BF16, name=f"eP1_{ph}", tag=f"eP1_{ph}"
                                )
                                nc.scalar.activation(
                                    eP1, pT1[:, 0:392],
                                    mybir.ActivationFunctionType.Exp,
                                )
                                # AV with ones-column denominator (av rows 0:64 =
                                # out, row 64 = softmax denominator)
                                for j2 in range(2):
                                    sl = 4 * q4 + 2 * ph + j2
                                    u = 2 * ph + j2
                                    nc.tensor.matmul(
                                        av[:, 512 * u : 512 * u + 196],
                                        vT[sl][0:128, 0:65],
                                        eP0[:, 196 * j2 : 196 * j2 + 196],
                                        start=True,
                                        stop=False,
                                    )
                                    nc.tensor.matmul(
                                        av[:, 512 * u : 512 * u + 196],
                                        vT[sl][0:68, 65:130],
                                        eP1[:, 196 * j2 : 196 * j2 + 196],
                                        start=False,
                                        stop=True,
                                    )
                                # per-pair denominator chain (keeps the tail
                                # short so the next head's phase A can start)
                                den2 = work.tile([1, 392], F32, name=f"den2_{ph}",
                                                 tag=f"den2_{ph}")
                                nc.scalar.copy(
                                    den2.rearrange("p (a c) -> p a c", a=2),
                                    av[64:65, 1024 * ph : 1024 * ph + 1024]
                                    .rearrange("p (a c) -> p a c", a=2)[:, :, 0:196],
                                )
                                rcp = work.tile([1, 392], F32, name=f"rcp_{ph}",
                                                tag=f"rcp_{ph}")
                                nc.vector.reciprocal_approx_fast(rcp, den2)
                                for j2 in range(2):
                                    sl = 4 * q4 + 2 * ph + j2
                                    u = 2 * ph + j2
                                    p, e = sl // 2, sl % 2
                                    bc = work.tile([64, N], F32, name="bc", tag="bc")
                                    nc.gpsimd.partition_broadcast(
                                        bc, rcp[0:1, 196 * j2 : 196 * j2 + 196]
                                    )
                                    avs = av[0:64, 512 * u : 512 * u + 196]
                                    fsl = featp[p][0:64, 196 * e : 196 * e + 196]
                                    rc = (rcat01 if i < 2 else rcat23)[p][
                                        64 * (i % 2) : 64 * (i % 2) + 64,
                                        196 * e : 196 * e + 196,
                                    ]
                                    if i < NHEADS - 1:
                                        nc.vector.tensor_mul(fsl, avs, bc)
                                        nc.vector.tensor_scalar_max(rc, fsl, 0.0)
                                        nc.vector.tensor_add(
                                            fsl, fsl,
                                            x_sb[p][:, 196 * e : 196 * e + 196],
                                        )
                                    else:
                                        nc.vector.scalar_tensor_tensor(
                                            rc,
                                            avs,
                                            0.0,
                                            bc,
                                            op0=mybir.AluOpType.max,
                                            op1=mybir.AluOpType.mult,
                                        )

                # ---- projection + output ----
                with tc.tile_pool(name=f"psP{b}", bufs=2, space="PSUM") as pP:
                    for p in range(npair):
                        for m in range(2):
                            op = pP.tile([128, 392], F32, name=f"op{p}_{m}", tag=f"op{m}")
                            nc.tensor.matmul(
                                op,
                                pw0[:, 128 * m : 128 * m + 128],
                                rcat01[p],
                                start=True,
                                stop=False,
                            )
                            nc.tensor.matmul(
                                op,
                                pw1[:, 128 * m : 128 * m + 128],
                                rcat23[p],
                                start=False,
                                stop=True,
                            )
                            ob = outp.tile([128, 392], F32, name=f"ob{m}", tag=f"ob{m}")
                            if m == 0:
                                nc.scalar.activation(
                                    ob, op, mybir.ActivationFunctionType.Identity,
                                    bias=pb0, scale=1.0,
                                )
                            else:
                                nc.vector.tensor_scalar_add(ob, op, pb1)
                            nc.sync.dma_start(
                                out=out_d[
                                    s0 + 2 * p : s0 + 2 * p + 2,
                                    128 * m : 128 * m + 128,
                                    :,
                                ].rearrange("s o n -> o s n"),
                                in_=ob.rearrange("o (s n) -> o s n", s=2),
                            )
    nc.finalize()
    return nc


_CACHE = {}


def _get_nc():
    if "nc" not in _CACHE:
        _CACHE["nc"] = build_bass()
    return _CACHE["nc"]


def _make_in_maps(inputs):
    import ml_dtypes

    host = _prep_host(inputs)
    x = (
        np.asarray(inputs["x"], np.float32)
        .reshape(BATCH, DIM, N)
        .astype(ml_dtypes.bfloat16)
    )
    in_maps = []
    for c in range(NCORES):
        m = {"x": np.ascontiguousarray(x[c * SPC : (c + 1) * SPC])}
        m.update(host)
        in_maps.append(m)
    return in_maps


def kernel(**inputs) -> np.ndarray:
    from concourse.bass_utils import run_bass_kernel_spmd

    nc = _get_nc()
    in_maps = _make_in_maps(inputs)
    res = run_bass_kernel_spmd(nc, in_maps, list(range(NCORES)))
    out = np.concatenate([r["out"] for r in res.results], axis=0)
    return out.reshape(BATCH, DIM, RES, RES).astype(np.float32)



# revision 16
# speedup vs baseline: 1.1743x; 1.1743x over previous
"""CascadedGroupAttention Trainium2 kernel.

Data-parallel over batch: B=512 split as 64 samples x 8 cores. Inside each
core a fully fused per-head cascade runs phase-major over sample blocks.

Key restructurings vs the reference:
  - qkv BN affine folded into matmul weights; bias applied via a ones-row
    appended to the feat operand (K=65).
  - softmax 1/sqrt(d) scale folded into the k weights.
  - depthwise 5x5 conv computed on the tensor engine as 25 PSUM-accumulated
    matmuls with [k;q]-interleaved diagonal weight matrices and
    edge-trimmed access patterns (no padding, no im2col).
  - attention computed transposed (P^T = K^T Q + ab^T) so softmax needs no
    transposes: raw exp is safe (logits bounded ~[-9, 10]), the denominator
    comes from a ones-column appended to v^T in the AV matmul, and the
    1/denom broadcast across partitions is done by gpsimd.
  - relative-position bias added by an identity-weight matmul accumulating
    onto the QK PSUM bank (ab is symmetric, so ab^T = ab).
  - relu'd head outputs collected in bf16; the output projection runs in
    bf16 with its BN affine folded into weights/final eviction bias.
"""

import os
import sys

import numpy as np

sys.path.insert(0, "/opt/trn_rl_repo")

import concourse.bass as bass  # noqa: E402
from concourse import bacc  # noqa: E402
import concourse.mybir as mybir  # noqa: E402
from concourse.masks import make_identity  # noqa: E402
from concourse.tile import TileContext  # noqa: E402

F32 = mybir.dt.float32
BF16 = mybir.dt.bfloat16

NHEADS = 4
KD = 16          # key dim
DV = 64          # per-head value dim
CH = 64          # per-head input channels (dim // heads)
RES = 14
N = RES * RES    # 196 tokens
DIM = 256
BATCH = 512
NCORES = 8
SPC = BATCH // NCORES   # samples per core = 64
BLK = 16                # samples per pipeline block
SCALE = KD ** -0.5

# tap order: center first so the first conv matmul covers the full output
# region (start=True then has full has_written coverage for accumulation)
TAPS = [(0, 0)] + [
    (dr, dc) for dr in range(-2, 3) for dc in range(-2, 3) if (dr, dc) != (0, 0)
]


def _prep_host(inp):
    """Fold BN affines into weights and build hardware-layout arrays."""
    import ml_dtypes

    bf16 = ml_dtypes.bfloat16
    qkv_w = np.asarray(inp["qkv_w"], np.float32)
    qkv_scale = np.asarray(inp["qkv_scale"], np.float32)
    qkv_bias = np.asarray(inp["qkv_bias"], np.float32)
    dw_w = np.asarray(inp["dw_w"], np.float32)
    dw_scale = np.asarray(inp["dw_scale"], np.float32)
    dw_bias = np.asarray(inp["dw_bias"], np.float32)
    proj_w = np.asarray(inp["proj_w"], np.float32)
    proj_scale = np.asarray(inp["proj_scale"], np.float32)
    proj_bias = np.asarray(inp["proj_bias"], np.float32)
    ab_full = np.asarray(inp["attention_biases"], np.float32)[
        :, np.asarray(inp["bias_idxs"])
    ]  # [4, 196, 196], symmetric in (n, m)

    # fused q+v per-head weight: cols 0:16 q, 16:64 zero, 64:128 v (v at 64
    # so its PE transpose reads a legal 64-aligned base). k separate, M=32
    # (cols 16:32 zero), col-tiled into 32-row slots as in the 4-sample
    # scheme. Row 64 carries the folded BN bias (ones-row trick).
    w_qv = np.zeros((NHEADS, 65, 128), np.float32)
    w_k = np.zeros((NHEADS, 65, 32), np.float32)
    conv_diag = np.zeros((NHEADS, 25, 128, 128), np.float32)
    dwb_pat = np.zeros((NHEADS, 128, 1), np.float32)
    for i in range(NHEADS):
        for j in range(KD):
            w_qv[i, :CH, j] = qkv_w[i, j] * qkv_scale[i, j]
            w_qv[i, CH, j] = qkv_bias[i, j]
            w_k[i, :CH, j] = qkv_w[i, KD + j] * qkv_scale[i, KD + j] * SCALE
            w_k[i, CH, j] = qkv_bias[i, KD + j] * SCALE
        for d in range(DV):
            w_qv[i, :CH, 64 + d] = qkv_w[i, 2 * KD + d] * qkv_scale[i, 2 * KD + d]
            w_qv[i, CH, 64 + d] = qkv_bias[i, 2 * KD + d]
        for t, (dr, dc) in enumerate(TAPS):
            for p in range(128):
                c = p % 32
                if c < 16:  # q channels sit in rows 32j..32j+15
                    conv_diag[i, t, p, p] = dw_w[i, c, dr + 2, dc + 2] * dw_scale[i, c]
        for p in range(128):
            c = p % 32
            if c < 16:
                dwb_pat[i, p, 0] = dw_bias[i, c]
    # scatter: route q (rows 0:16 of a qv pair tile) to rows 32j..32j+16
    sctr32 = np.zeros((16, 4, 128), np.float32)
    for c in range(16):
        for j in range(4):
            sctr32[c, j, 32 * j + c] = 1.0

    proj_wT = np.ascontiguousarray(
        (proj_w * proj_scale[:, None]).T
    )  # [cat_c, out_o]
    pw_bf = proj_wT.astype(bf16)
    pb = np.ascontiguousarray(proj_bias.reshape(2, 128, 1).astype(np.float32))

    return {
        "w_qv": w_qv.astype(bf16),
        "w_k": w_k.astype(bf16),
        "sctr32": sctr32.astype(bf16),
        "conv_diag": np.ascontiguousarray(conv_diag.astype(bf16)),
        "dwb_pat": dwb_pat,
        "ab": np.ascontiguousarray(ab_full.astype(bf16)),
        "proj_wT": pw_bf,
        "proj_b": pb,
    }


def build_bass(spc=SPC, blk=BLK):
    nc = bacc.Bacc(None, target_bir_lowering=False)

    x_d = nc.declare_dram_parameter("x", [spc, DIM, N], BF16, isOutput=False)
    wqv_d = nc.declare_dram_parameter("w_qv", [NHEADS, 65, 128], BF16, isOutput=False)
    wk_d = nc.declare_dram_parameter("w_k", [NHEADS, 65, 32], BF16, isOutput=False)
    sctr_d = nc.declare_dram_parameter("sctr32", [16, 4, 128], BF16, isOutput=False)
    cdiag_d = nc.declare_dram_parameter(
        "conv_diag", [NHEADS, 25, 128, 128], BF16, isOutput=False
    )
    dwb_d = nc.declare_dram_parameter("dwb_pat", [NHEADS, 128, 1], F32, isOutput=False)
    ab_d = nc.declare_dram_parameter("ab", [NHEADS, N, N], BF16, isOutput=False)
    pw_d = nc.declare_dram_parameter("proj_wT", [DIM, DIM], BF16, isOutput=False)
    pb_d = nc.declare_dram_parameter("proj_b", [2, 128, 1], F32, isOutput=False)
    out_d = nc.declare_dram_parameter("out", [spc, DIM, N], F32, isOutput=True)

    nblk = spc // blk
    npair = blk // 2

    with TileContext(nc) as tc:
        with (
            tc.tile_pool(name="const", bufs=1) as constp,
            tc.tile_pool(name="persist", bufs=1) as persist,
            tc.tile_pool(name="work", bufs=3) as work,
            tc.tile_pool(name="outp", bufs=4) as outp,
        ):
            # ---- constants ----
            ident = constp.tile([128, 128], BF16, name="ident")
            make_identity(nc, ident)
            sctr_sb = constp.tile([16, 4, 128], BF16, name="sctr")
            nc.sync.dma_start(out=sctr_sb, in_=sctr_d[0:16])
            wqv_sb = []
            wk_sb = []
            dwb_sb = []
            ab0_sb = []
            ab1_sb = []
            for i in range(NHEADS):
                t = constp.tile([65, 128], BF16, name=f"wqv{i}", tag=f"wqv{i}")
                nc.sync.dma_start(out=t, in_=wqv_d[i])
                wqv_sb.append(t)
                t = constp.tile([65, 32], BF16, name=f"wk{i}", tag=f"wk{i}")
                nc.sync.dma_start(out=t, in_=wk_d[i])
                wk_sb.append(t)
                t = constp.tile([128, 1], F32, name=f"dwb{i}", tag=f"dwb{i}")
                nc.sync.dma_start(out=t, in_=dwb_d[i])
                dwb_sb.append(t)
                t = constp.tile([128, 2, N], BF16, name=f"ab0_{i}", tag=f"ab0_{i}")
                nc.sync.dma_start(out=t[:, 0, :], in_=ab_d[i, 0:128, :])
                nc.sync.dma_start(out=t[:, 1, :], in_=ab_d[i, 0:128, :])
                ab0_sb.append(t)
                t = constp.tile([68, 2, N], BF16, name=f"ab1_{i}", tag=f"ab1_{i}")
                nc.sync.dma_start(out=t[:, 0, :], in_=ab_d[i, 128:196, :])
                nc.sync.dma_start(out=t[:, 1, :], in_=ab_d[i, 128:196, :])
                ab1_sb.append(t)
            cdg_sb = []
            for i in range(NHEADS):
                t = constp.tile([128, 25, 128], BF16, name=f"cdg{i}", tag=f"cdg{i}")
                nc.sync.dma_start(out=t, in_=cdiag_d[i].rearrange("t p c -> p t c"))
                cdg_sb.append(t)
            pw0 = constp.tile([128, DIM], BF16, name="pw0")
            nc.sync.dma_start(out=pw0, in_=pw_d[0:128, :])
            pw1 = constp.tile([128, DIM], BF16, name="pw1")
            nc.sync.dma_start(out=pw1, in_=pw_d[128:256, :])
            pb0 = constp.tile([128, 1], F32, name="pb0")
            nc.sync.dma_start(out=pb0, in_=pb_d[0])
            pb1 = constp.tile([128, 1], F32, name="pb1")
            nc.sync.dma_start(out=pb1, in_=pb_d[1])

            for b in range(min(nblk, int(os.environ.get('MAXBLK', '99')))):
                s0 = b * blk
                # ---- per-block persistent tiles (sample pairs) ----
                featp = []
                for p in range(npair):
                    t = persist.tile(
                        [65, 392], BF16, name=f"feat{b}_{p}", tag=f"feat{p}", bufs=2
                    )
                    nc.sync.dma_start(
                        out=t[0:CH, :].rearrange("c (s n) -> c s n", s=2),
                        in_=x_d[s0 + 2 * p : s0 + 2 * p + 2, 0:CH, :].rearrange(
                            "s c n -> c s n"
                        ),
                    )
                    nc.gpsimd.memset(t[CH : CH + 1, :], 1.0)
                    featp.append(t)
                rcat01 = []
                rcat23 = []
                for p in range(npair):
                    rcat01.append(
                        persist.tile(
                            [128, 392], BF16, name=f"rA{b}_{p}", tag=f"rA{p}", bufs=2
                        )
                    )
                    rcat23.append(
                        persist.tile(
                            [128, 392], BF16, name=f"rB{b}_{p}", tag=f"rB{p}", bufs=2
                        )
                    )

                for i in range(NHEADS):
                    # prefetch next head's x chunk (pair layout)
                    x_sb = None
                    if i < NHEADS - 1:
                        x_sb = []
                        for p in range(npair):
                            t = work.tile(
                                [CH, 392],
                                BF16,
                                name=f"x{b}_{i}_{p}",
                                tag=f"x{p}",
                                bufs=2,
                            )
                            nc.sync.dma_start(
                                out=t.rearrange("c (s n) -> c s n", s=2),
                                in_=x_d[
                                    s0 + 2 * p : s0 + 2 * p + 2,
                                    (i + 1) * CH : (i + 2) * CH,
                                    :,
                                ].rearrange("s c n -> c s n"),
                            )
                            x_sb.append(t)

                    # ================= phase A =================
                    # per pair: one fused q+v matmul (q rows 0:16, v rows
                    # 64:128); k col-tiled 4 pairs to a bank at rows 32j.
                    qv = []    # per-pair SBUF [128, 392] bf16
                    kf = []    # per-half-block SBUF [128, 392] bf16 (4 pairs)
                    qf = []    # per-half-block conv'd q, same layout as kf
                    vT = []    # per-sample transposed v [128, 130]
                    with tc.tile_pool(name=f"psA{b}_{i}", bufs=1, space="PSUM") as pA:
                        for p in range(npair):
                            qvp = pA.tile([128, 392], F32, name=f"qv{p}", tag="qv",
                                          bufs=2)
                            nc.tensor.matmul(
                                qvp, wqv_sb[i], featp[p], start=True, stop=True
                            )
                            t = persist.tile(
                                [128, 392], BF16, name=f"qv{b}_{i}_{p}",
                                tag=f"qv{p}", bufs=2,
                            )
                            if p % 2 == 0:
                                nc.scalar.copy(t, qvp)
                            else:
                                nc.vector.tensor_copy(t, qvp)
                            qv.append(t)

                        # k matmuls: bank t holds pairs 4t..4t+3 at rows 32j
                        for th in range(2):
                            kp = pA.tile([128, 392], F32, name=f"kp{th}", tag="kp",
                                         bufs=2)
                            for j in range(4):
                                nc.tensor.matmul(
                                    kp[32 * j : 32 * j + 32, :],
                                    wk_sb[i],
                                    featp[4 * th + j],
                                    start=True,
                                    stop=True,
                                    tile_position=(0, 32 * j),
                                )
                            t = persist.tile(
                                [128, 392], BF16, name=f"kf{b}_{i}_{th}",
                                tag=f"kf{th}", bufs=2,
                            )
                            if th == 0:
                                nc.scalar.copy(t, kp)
                            else:
                                nc.vector.tensor_copy(t, kp)
                            kf.append(t)

                        # v transposes (row-groups 2-3) overlap the q scatter
                        for p in range(npair):
                            for e in range(2):
                                sl = 2 * p + e
                                vTp = pA.tile([128, 128], BF16, name=f"vTp{sl}",
                                              tag="vTp", bufs=1)
                                nc.tensor.transpose(
                                    vTp[0:128, 0:64],
                                    qv[p][64:128, 196 * e : 196 * e + 128],
                                    ident[64:128, 64:128],
                                )
                                nc.tensor.transpose(
                                    vTp[0:68, 64:128],
                                    qv[p][64:128, 196 * e + 128 : 196 * e + 196],
                                    ident[64:128, 64:128],
                                )
                                vT_t = persist.tile(
                                    [128, 130], BF16, name=f"vT{b}_{i}_{sl}",
                                    tag=f"vT{sl}", bufs=2,
                                )
                                nc.vector.tensor_copy(vT_t[:, 0:64], vTp[:, 0:64])
                                nc.vector.tensor_copy(
                                    vT_t[0:68, 65:129], vTp[0:68, 64:128]
                                )
                                nc.gpsimd.memset(vT_t[:, 64:65], 1.0)
                                nc.gpsimd.memset(vT_t[:, 129:130], 1.0)
                                vT.append(vT_t)

                        # scatter q into conv layout (rows 32j, pairs 4t..4t+3)
                        # then 8-sample packed conv per half-block
                        for th in range(2):
                            qs = pA.tile([128, 392], F32, name=f"qs{th}", tag="qs",
                                         bufs=2)
                            for j in range(4):
                                nc.tensor.matmul(
                                    qs,
                                    sctr_sb[:, j, :],
                                    qv[4 * th + j][0:16, :],
                                    start=(j == 0),
                                    stop=(j == 3),
                                )
                            qpad = work.tile([128, 648], BF16, name=f"qpad{th}",
                                             tag=f"qpad{th}")
                            nc.gpsimd.memset(qpad, 0.0)
                            qp4 = qpad.rearrange("p (h r c) -> p h r c", h=2, c=18)
                            nc.scalar.copy(
                                qp4[:, :, 2:16, 2:16],
                                qs.rearrange("p (h r c) -> p h r c", h=2, c=RES),
                            )
                            dqp = pA.tile([128, 392], F32, name=f"dqp{th}",
                                          tag="dqp", bufs=1)
                            for t_, (dr, dc) in enumerate(TAPS):
                                nc.tensor.matmul(
                                    dqp,
                                    cdg_sb[i][:, t_, :],
                                    qp4[:, :, 2 + dr : 16 + dr, 2 + dc : 16 + dc],
                                    start=(t_ == 0),
                                    stop=(t_ == len(TAPS) - 1),
                                )
                            g8 = work.tile([128, 392], BF16, name=f"g8{th}",
                                           tag="g8")
                            nc.scalar.activation(
                                g8, dqp, mybir.ActivationFunctionType.Gelu,
                                bias=dwb_sb[i], scale=1.0,
                            )
                            qf_t = persist.tile([128, 392], BF16,
                                                name=f"qf{b}_{i}_{th}",
                                                tag=f"qf{th}", bufs=2)
                            nc.vector.tensor_add(
                                qf_t.rearrange("p (h r c) -> p h r c", h=2, c=RES),
                                g8.rearrange("p (h r c) -> p h r c", h=2, c=RES),
                                qp4[:, :, 2:16, 2:16],
                            )
                            qf.append(qf_t)

                    # ================= phase B =================
                    # Order per q4: QK(ph0), exp(ph0), QK(ph1), exp(ph1),
                    # AV(ph0), AV(ph1), then the denominator chains — so the
                    # tensor engine never stalls waiting on an exp, and the
                    # chains drain while the next q4's QKs run.
                    with tc.tile_pool(name=f"psB{b}_{i}", bufs=1, space="PSUM") as pB:
                        for q4 in range(blk // 4):  # 4 samples at a time
                            eP0s, eP1s, avs_t = [], [], []
                            for ph in range(2):
                                pT0 = pB.tile(
                                    [128, 512], F32, name=f"pT0_{ph}", tag="pT0",
                                    bufs=2,
                                )
                                pT1 = pB.tile(
                                    [68, 512], F32, name=f"pT1_{ph}", tag="pT1",
                                    bufs=2,
                                )
                                # start=True clears has_written for the WHOLE
                                # bank, so each sample's QK+ab group completes
                                # before the next sample's QK starts.
                                for j2 in range(2):
                                    sl = 4 * q4 + 2 * ph + j2
                                    p, e = sl // 2, sl % 2
                                    th, j = p // 4, p % 4
                                    kf_s = kf[th][
                                        32 * j : 32 * j + 16,
                                        196 * e : 196 * e + 196,
                                    ]
                                    qf_s = qf[th][
                                        32 * j : 32 * j + 16,
                                        196 * e : 196 * e + 196,
                                    ]
                                    off = 196 * j2
                                    nc.tensor.matmul(
                                        pT0[:, off : off + 196],
                                        kf_s[:, 0:128],
                                        qf_s,
                                        start=True,
                                        stop=False,
                                        tile_position=(32 * j, 0),
                                    )
                                    nc.tensor.matmul(
                                        pT1[:, off : off + 196],
                                        kf_s[:, 128:196],
                                        qf_s,
                                        start=True,
                                        stop=False,
                                        tile_position=(32 * j, 0),
                                    )
                                    nc.tensor.matmul(
                                        pT0[:, off : off + 196],
                                        ident,
                                        ab0_sb[i][:, j2, :],
                                        start=False,
                                        stop=True,
                                    )
                                    nc.tensor.matmul(
                                        pT1[:, off : off + 196],
                                        ident[0:68, 0:68],
                                        ab1_sb[i][:, j2, :],
                                        start=False,
                                        stop=True,
                                    )
                                eP0 = work.tile(
                                    [128, 392], BF16, name=f"eP0_{ph}", tag=f"eP0_{ph}"
                                )
                                nc.scalar.activation(
                                    eP0, pT0[:, 0:392],
                                    mybir.ActivationFunctionType.Exp,
                                )
                                eP1 = work.tile(
                                    [68, 392], BF16, name=f"eP1_{ph}", tag=f"eP1_{ph}"
                                )
                                nc.scalar.activation(
                                    eP1, pT1[:, 0:392],
                                    mybir.ActivationFunctionType.Exp,
                                )
                                eP0s.append(eP0)
                                eP1s.append(eP1)

                            for ph in range(2):
                                av = pB.tile([65, 1024], F32, name=f"av{q4}_{ph}",
                                             tag="av", bufs=2)
                                avs_t.append(av)
                                for j2 in range(2):
                                    sl = 4 * q4 + 2 * ph + j2
                                    nc.tensor.matmul(
                                        av[:, 512 * j2 : 512 * j2 + 196],
                                        vT[sl][0:128, 0:65],
                                        eP0s[ph][:, 196 * j2 : 196 * j2 + 196],
                                        start=True,
                                        stop=False,
                                    )
                                    nc.tensor.matmul(
                                        av[:, 512 * j2 : 512 * j2 + 196],
                                        vT[sl][0:68, 65:130],
                                        eP1s[ph][:, 196 * j2 : 196 * j2 + 196],
                                        start=False,
                                        stop=True,
                                    )

                            # denominator chains: one Reciprocal ACT straight
                            # off the PSUM den row, then broadcast + normalize
                            for ph in range(2):
                                av = avs_t[ph]
                                den2 = work.tile([1, 392], F32, name=f"den2_{ph}",
                                                 tag=f"den2_{ph}")
                                nc.scalar.copy(
                                    den2.rearrange("p (a c) -> p a c", a=2),
                                    av[64:65, :].rearrange(
                                        "p (a c) -> p a c", a=2)[:, :, 0:196],
                                )
                                rcp = work.tile([1, 392], F32, name=f"rcp_{ph}",
                                                tag=f"rcp_{ph}")
                                nc.vector.reciprocal_approx_fast(rcp, den2)
                                for j2 in range(2):
                                    sl = 4 * q4 + 2 * ph + j2
                                    p, e = sl // 2, sl % 2
                                    bc = work.tile([64, N], F32, name="bc", tag="bc")
                                    nc.gpsimd.partition_broadcast(
                                        bc, rcp[0:1, 196 * j2 : 196 * j2 + 196]
                                    )
                                    avs = av[0:64, 512 * j2 : 512 * j2 + 196]
                                    fsl = featp[p][0:64, 196 * e : 196 * e + 196]
                                    rc = (rcat01 if i < 2 else rcat23)[p][
                                        64 * (i % 2) : 64 * (i % 2) + 64,
                                        196 * e : 196 * e + 196,
                                    ]
                                    if i < NHEADS - 1:
                                        nc.vector.tensor_mul(fsl, avs, bc)
                                        nc.vector.tensor_scalar_max(rc, fsl, 0.0)
                                        nc.vector.tensor_add(
                                            fsl, fsl,
                                            x_sb[p][:, 196 * e : 196 * e + 196],
                                        )
                                    else:
                                        nc.vector.scalar_tensor_tensor(
                                            rc,
                                            avs,
                                            0.0,
                                            bc,
                                            op0=mybir.AluOpType.max,
                                            op1=mybir.AluOpType.mult,
                                        )

                # ---- projection + output ----
                with tc.tile_pool(name=f"psP{b}", bufs=2, space="PSUM") as pP:
                    for p in range(npair):
                        for m in range(2):
                            op = pP.tile([128, 392], F32, name=f"op{p}_{m}", tag=f"op{m}")
                            nc.tensor.matmul(
                                op,
                                pw0[:, 128 * m : 128 * m + 128],
                                rcat01[p],
                                start=True,
                                stop=False,
                            )
                            nc.tensor.matmul(
                                op,
                                pw1[:, 128 * m : 128 * m + 128],
                                rcat23[p],
                                start=False,
                                stop=True,
                            )
                            ob = outp.tile([128, 392], F32, name=f"ob{m}", tag=f"ob{m}")
                            if m == 0:
                                nc.scalar.activation(
                                    ob, op, mybir.ActivationFunctionType.Identity,
                                    bias=pb0, scale=1.0,
                                )
                            else:
                                nc.vector.tensor_scalar_add(ob, op, pb1)
                            nc.sync.dma_start(
                                out=out_d[
                                    s0 + 2 * p : s0 + 2 * p + 2,
                                    128 * m : 128 * m + 128,
                                    :,
                                ].rearrange("s o n -> o s n"),
                                in_=ob.rearrange("o (s n) -> o s n", s=2),
                            )
    nc.finalize()
    return nc


_CACHE = {}


def _get_nc():
    if "nc" not in _CACHE:
        _CACHE["nc"] = build_bass()
    return _CACHE["nc"]


def _make_in_maps(inputs):
    import ml_dtypes

    host = _prep_host(inputs)
    x = (
        np.asarray(inputs["x"], np.float32)
        .reshape(BATCH, DIM, N)
        .astype(ml_dtypes.bfloat16)
    )
    in_maps = []
    for c in range(NCORES):
        m = {"x": np.ascontiguousarray(x[c * SPC : (c + 1) * SPC])}
        m.update(host)
        in_maps.append(m)
    return in_maps


def kernel(**inputs) -> np.ndarray:
    from concourse.bass_utils import run_bass_kernel_spmd

    nc = _get_nc()
    in_maps = _make_in_maps(inputs)
    res = run_bass_kernel_spmd(nc, in_maps, list(range(NCORES)))
    out = np.concatenate([r["out"] for r in res.results], axis=0)
    return out.reshape(BATCH, DIM, RES, RES).astype(np.float32)

